# revision 1
# baseline (speedup 1.0000x reference)
"""MinGRU (B=4, T=4096, D=1024) Trainium2 kernel, 8-core SPMD.

Sharding: core i handles (batch b = i//2, output-channel half j = i%2).
Each core computes u_z = x[b] @ Wz[half].T, u_h = x[b] @ Wh[half].T,
z = sigmoid(u_z + bz), a = 1 - z + 1e-8, bvec = z * (u_h + bh), then the
recurrence h_t = a_t * h_{t-1} + b_t via the hardware tensor_tensor_scan.

Host pre-transposes x and W so every DMA is row-contiguous, and
re-transposes the per-core (512, 4096) results into the full output.
"""

import numpy as np

_B, _T, _D = 4, 4096, 1024
_EH = 512          # output channels per core
_NG = _EH // 128   # 4 channel groups of 128 partitions
_TT = 512          # timestep tile (one PSUM bank, max fp32 moving free dim)
_NT = _T // _TT    # 8 t-tiles
_NK = _D // 128    # 8 contraction tiles

_EPS = 1e-8


def _build(reps=1, loop_n=None):
    from contextlib import ExitStack
    from concourse import bacc, mybir, tile

    f32 = mybir.dt.float32
    f32r = mybir.dt.float32r
    AF = mybir.ActivationFunctionType
    OP = mybir.AluOpType

    nc = bacc.Bacc("TRN2", debug=False, num_devices=8)
    xt = nc.dram_tensor("xt", [_D, _T], f32, kind="ExternalInput").ap()
    wzt = nc.dram_tensor("wzt", [_D, _EH], f32, kind="ExternalInput").ap()
    wht = nc.dram_tensor("wht", [_D, _EH], f32, kind="ExternalInput").ap()
    bzt = nc.dram_tensor("bzt", [128, _NG], f32, kind="ExternalInput").ap()
    bht = nc.dram_tensor("bht", [128, _NG], f32, kind="ExternalInput").ap()
    hout = nc.dram_tensor("h", [_EH, _T], f32, kind="ExternalOutput").ap()

    with tile.TileContext(nc) as tc, ExitStack() as ctx:
        wpool = ctx.enter_context(tc.tile_pool(name="w", bufs=1))
        xpool = ctx.enter_context(tc.tile_pool(name="x", bufs=3))
        vpool = ctx.enter_context(tc.tile_pool(name="v", bufs=3))
        hpool = ctx.enter_context(tc.tile_pool(name="h", bufs=2))
        ppool = ctx.enter_context(tc.tile_pool(name="p", bufs=3, space="PSUM"))
        plast = ctx.enter_context(tc.tile_pool(name="pl", bufs=1, space="PSUM"))

        def load_x(t, n_chunks=2):
            # x^T tile for this t-slice: [128, (k tt)], chunked k-block DMAs.
            xs = xpool.tile([128, _NK * _TT], f32r, tag="x")
            step = _NK // n_chunks
            for c in range(n_chunks):
                ks = c * step
                nc.sync.dma_start(
                    xs[:, ks * _TT:(ks + step) * _TT].rearrange(
                        "p (k t) -> p k t", k=step),
                    xt.rearrange("(k p) t -> p k t", p=128)[
                        :, ks:ks + step, t * _TT:(t + 1) * _TT].bitcast(f32r),
                )
            return xs

        # Startup DMA order matters: the first matmuls need x chunk0 and the
        # first weight k-tiles; interleave x chunks with per-k weight tiles
        # so PE starts ~2-3us in and stays fed while the rest streams.
        # Weights resident in SBUF: [128, (k e)] so lhsT k/g tiles are slices.
        # float32r tiles (same bytes as f32; PE truncates to FP22 on read):
        # the BIR verifier requires fp32r-matmul inputs be produced as fp32r.
        xs0 = xpool.tile([128, _NK * _TT], f32r, tag="x")
        wz_sb = wpool.tile([128, _NK * _EH], f32r, tag="wz")
        wh_sb = wpool.tile([128, _NK * _EH], f32r, tag="wh")
        bz_sb = wpool.tile([128, _NG], f32, tag="bz")
        bh_sb = wpool.tile([128, _NG], f32, tag="bh")

        def x0_chunk(ks, nk):
            nc.sync.dma_start(
                xs0[:, ks * _TT:(ks + nk) * _TT].rearrange(
                    "p (k t) -> p k t", k=nk),
                xt.rearrange("(k p) t -> p k t", p=128)[
                    :, ks:ks + nk, 0:_TT].bitcast(f32r),
            )

        def w_chunk(k):
            nc.sync.dma_start(
                wz_sb[:, k * _EH:(k + 1) * _EH],
                wzt[k * 128:(k + 1) * 128, :].bitcast(f32r),
            )
            nc.sync.dma_start(
                wh_sb[:, k * _EH:(k + 1) * _EH],
                wht[k * 128:(k + 1) * 128, :].bitcast(f32r),
            )

        x0_chunk(0, 2)
        w_chunk(0)
        w_chunk(1)
        nc.sync.dma_start(bz_sb[:], bzt)
        nc.sync.dma_start(bh_sb[:], bht)
        x0_chunk(2, 2)
        w_chunk(2)
        w_chunk(3)
        x0_chunk(4, 2)
        w_chunk(4)
        w_chunk(5)
        x0_chunk(6, 2)
        w_chunk(6)
        w_chunk(7)

        def body(first):
          hprev = [None] * _NG
          for t in range(_NT):
            xs = xs0 if (first and t == 0) else load_x(t)
            for g in range(_NG):
                last = (t == _NT - 1 and g == _NG - 1)
                # Final group runs as two half-width pipelines so the
                # ACT/DVE/DMA epilogue drains during the second half's
                # matmuls instead of entirely after the last one.
                halves = ((0, _TT // 2), (_TT // 2, _TT // 2)) if last \
                    else ((0, _TT),)
                prev_ap = None if t == 0 else hprev[g][:, _TT - 1:_TT]
                for (c0, w) in halves:
                    if last:
                        pz = plast.tile([128, w], f32, tag="pzl")
                        ph = plast.tile([128, w], f32, tag="phl")
                    else:
                        pz = ppool.tile([128, w], f32, tag="pz")
                        ph = ppool.tile([128, w], f32, tag="ph")
                    for k in range(_NK):
                        nc.tensor.matmul(
                            pz[:],
                            lhsT=wz_sb[:, k * _EH + g * 128: k * _EH + (g + 1) * 128],
                            rhs=xs[:, k * _TT + c0: k * _TT + c0 + w],
                            start=(k == 0),
                            stop=(k == _NK - 1),
                        )
                    for k in range(_NK):
                        nc.tensor.matmul(
                            ph[:],
                            lhsT=wh_sb[:, k * _EH + g * 128: k * _EH + (g + 1) * 128],
                            rhs=xs[:, k * _TT + c0: k * _TT + c0 + w],
                            start=(k == 0),
                            stop=(k == _NK - 1),
                        )
                    z = vpool.tile([128, w], f32, tag="z")
                    nc.scalar.activation(z[:], pz[:], AF.Sigmoid,
                                         bias=bz_sb[:, g:g + 1])
                    av = vpool.tile([128, w], f32, tag="a")
                    nc.scalar.activation(av[:], z[:], AF.Copy,
                                         bias=1.0 + _EPS, scale=-1.0)
                    bv = vpool.tile([128, w], f32, tag="b")
                    nc.vector.scalar_tensor_tensor(
                        bv[:], ph[:], bh_sb[:, g:g + 1], z[:], OP.add, OP.mult
                    )
                    hb = hpool.tile([128, w], f32, tag=f"h{g}")
                    init = 0.0 if prev_ap is None else prev_ap
                    nc.vector.tensor_tensor_scan(hb[:], av[:], bv[:], init,
                                                 OP.mult, OP.add)
                    prev_ap = hb[:, w - 1:w]
                    if not last:
                        hprev[g] = hb
                    nc.sync.dma_start(
                        hout[g * 128:(g + 1) * 128,
                             t * _TT + c0: t * _TT + c0 + w], hb[:]
                    )

        if loop_n is not None:
            body(True)
            from concourse import mybir as _mb
            with tc.For_i(0, loop_n, 1, hint_engines=(_mb.EngineType.PE, _mb.EngineType.SP, _mb.EngineType.DVE, _mb.EngineType.Activation)):
                body(False)
        else:
            for rep in range(reps):
                body(rep == 0)
    nc.compile()
    return nc


_NC_CACHE = None


def _shard_inputs(inputs):
    """Per-core input maps: host pre-transposes so device DMAs are contiguous."""
    x = np.asarray(inputs["x"], dtype=np.float32)
    Wz = np.asarray(inputs["Wz"], dtype=np.float32)
    bz = np.asarray(inputs["bz"], dtype=np.float32)
    Wh = np.asarray(inputs["Wh"], dtype=np.float32)
    bh = np.asarray(inputs["bh"], dtype=np.float32)

    wzT = np.ascontiguousarray(Wz.T)  # [d, e]
    whT = np.ascontiguousarray(Wh.T)

    in_maps = []
    for i in range(8):
        b, j = i // 2, i % 2
        sl = slice(j * _EH, (j + 1) * _EH)
        in_maps.append({
            "xt": np.ascontiguousarray(x[b].T),              # [D, T]
            "wzt": np.ascontiguousarray(wzT[:, sl]),         # [D, EH]
            "wht": np.ascontiguousarray(whT[:, sl]),         # [D, EH]
            "bzt": np.ascontiguousarray(bz[sl].reshape(_NG, 128).T),  # [128, NG]
            "bht": np.ascontiguousarray(bh[sl].reshape(_NG, 128).T),
        })
    return in_maps


def run(inputs, trace=False, tmpdir=None):
    """Shard inputs, run the SPMD kernel on 8 cores, reassemble full output.

    Returns (output ndarray (B, T, D) float32, BassKernelResults).
    """
    global _NC_CACHE
    from concourse.bass_utils import run_bass_kernel_spmd

    if _NC_CACHE is None:
        _NC_CACHE = _build()
    nc = _NC_CACHE

    in_maps = _shard_inputs(inputs)

    res = run_bass_kernel_spmd(
        nc, in_maps, core_ids=list(range(8)), trace=trace, tmpdir=tmpdir
    )

    out = np.empty((_B, _T, _D), dtype=np.float32)
    for i in range(8):
        b, j = i // 2, i % 2
        out[b, :, j * _EH:(j + 1) * _EH] = res.results[i]["h"].T
    return out, res


def kernel(**inputs):
    out, _ = run(inputs, trace=False)
    return out



# revision 5
# speedup vs baseline: 1.1804x; 1.1804x over previous
"""MinGRU (B=4, T=4096, D=1024) Trainium2 kernel, 8-core SPMD.

Sharding: core i handles (batch b = i//2, output-channel half j = i%2).
Each core computes u_z = x[b] @ Wz[half].T, u_h = x[b] @ Wh[half].T,
z = sigmoid(u_z + bz), a = 1 - z + 1e-8, bvec = z * (u_h + bh), then the
recurrence h_t = a_t * h_{t-1} + b_t via the hardware tensor_tensor_scan.

Host pre-transposes x and W so every DMA is row-contiguous, and
re-transposes the per-core (512, 4096) results into the full output.
"""

import numpy as np

_B, _T, _D = 4, 4096, 1024
_EH = 512          # output channels per core
_NG = _EH // 128   # 4 channel groups of 128 partitions
_TT = 512          # timestep tile (one PSUM bank, max fp32 moving free dim)
_NT = _T // _TT    # 8 t-tiles
_NK = _D // 128    # 8 contraction tiles

_EPS = 1e-8


def _build(reps=1, loop_n=None):
    from contextlib import ExitStack
    from concourse import bacc, mybir, tile

    f32 = mybir.dt.float32
    bf16 = mybir.dt.bfloat16
    AF = mybir.ActivationFunctionType
    OP = mybir.AluOpType

    nc = bacc.Bacc("TRN2", debug=False, num_devices=8)
    xt = nc.dram_tensor("xt", [_D, _T], bf16, kind="ExternalInput").ap()
    wzt = nc.dram_tensor("wzt", [_D, _EH], bf16, kind="ExternalInput").ap()
    wht = nc.dram_tensor("wht", [_D, _EH], bf16, kind="ExternalInput").ap()
    bzt = nc.dram_tensor("bzt", [128, _NG], f32, kind="ExternalInput").ap()
    bht = nc.dram_tensor("bht", [128, _NG], f32, kind="ExternalInput").ap()
    hout = nc.dram_tensor("h", [_EH, _T], f32, kind="ExternalOutput").ap()

    with tile.TileContext(nc) as tc, ExitStack() as ctx:
        wpool = ctx.enter_context(tc.tile_pool(name="w", bufs=1))
        xpool = ctx.enter_context(tc.tile_pool(name="x", bufs=3))
        vpool = ctx.enter_context(tc.tile_pool(name="v", bufs=3))
        hpool = ctx.enter_context(tc.tile_pool(name="h", bufs=2))
        ppool = ctx.enter_context(tc.tile_pool(name="p", bufs=3, space="PSUM"))
        plast = ctx.enter_context(tc.tile_pool(name="pl", bufs=1, space="PSUM"))

        def load_x(t, n_chunks=2):
            # x^T tile for this t-slice: [128, (k tt)], chunked k-block DMAs.
            xs = xpool.tile([128, _NK * _TT], bf16, tag="x")
            step = _NK // n_chunks
            for c in range(n_chunks):
                ks = c * step
                nc.sync.dma_start(
                    xs[:, ks * _TT:(ks + step) * _TT].rearrange(
                        "p (k t) -> p k t", k=step),
                    xt.rearrange("(k p) t -> p k t", p=128)[
                        :, ks:ks + step, t * _TT:(t + 1) * _TT],
                )
            return xs

        # Startup DMA order matters: the first matmuls need x chunk0 and the
        # first weight k-tiles; interleave x chunks with per-k weight tiles
        # so PE starts ~2-3us in and stays fed while the rest streams.
        # Weights resident in SBUF: [128, (k e)] so lhsT k/g tiles are slices.
        xs0 = xpool.tile([128, _NK * _TT], bf16, tag="x")
        wz_sb = wpool.tile([128, _NK * _EH], bf16, tag="wz")
        wh_sb = wpool.tile([128, _NK * _EH], bf16, tag="wh")
        bz_sb = wpool.tile([128, _NG], f32, tag="bz")
        bh_sb = wpool.tile([128, _NG], f32, tag="bh")

        def x0_chunk(ks, nk):
            nc.sync.dma_start(
                xs0[:, ks * _TT:(ks + nk) * _TT].rearrange(
                    "p (k t) -> p k t", k=nk),
                xt.rearrange("(k p) t -> p k t", p=128)[
                    :, ks:ks + nk, 0:_TT],
            )

        def w_chunk(k):
            nc.sync.dma_start(
                wz_sb[:, k * _EH:(k + 1) * _EH],
                wzt[k * 128:(k + 1) * 128, :],
            )
            nc.sync.dma_start(
                wh_sb[:, k * _EH:(k + 1) * _EH],
                wht[k * 128:(k + 1) * 128, :],
            )

        x0_chunk(0, 2)
        w_chunk(0)
        w_chunk(1)
        nc.sync.dma_start(bz_sb[:], bzt)
        nc.sync.dma_start(bh_sb[:], bht)
        x0_chunk(2, 2)
        w_chunk(2)
        w_chunk(3)
        x0_chunk(4, 2)
        w_chunk(4)
        w_chunk(5)
        x0_chunk(6, 2)
        w_chunk(6)
        w_chunk(7)

        def body(first):
          hprev = [None] * _NG
          for t in range(_NT):
            xs = xs0 if (first and t == 0) else load_x(t)
            for g in range(_NG):
                last = (t == _NT - 1 and g == _NG - 1)
                # Final group runs as two half-width pipelines so the
                # ACT/DVE/DMA epilogue drains during the second half's
                # matmuls instead of entirely after the last one.
                halves = ((0, _TT // 2), (_TT // 2, _TT // 2)) if last \
                    else ((0, _TT),)
                prev_ap = None if t == 0 else hprev[g][:, _TT - 1:_TT]
                for (c0, w) in halves:
                    if last:
                        pz = plast.tile([128, w], f32, tag="pzl")
                        ph = plast.tile([128, w], f32, tag="phl")
                    else:
                        pz = ppool.tile([128, w], f32, tag="pz")
                        ph = ppool.tile([128, w], f32, tag="ph")
                    for k in range(_NK):
                        nc.tensor.matmul(
                            pz[:],
                            lhsT=wz_sb[:, k * _EH + g * 128: k * _EH + (g + 1) * 128],
                            rhs=xs[:, k * _TT + c0: k * _TT + c0 + w],
                            start=(k == 0),
                            stop=(k == _NK - 1),
                        )
                    for k in range(_NK):
                        nc.tensor.matmul(
                            ph[:],
                            lhsT=wh_sb[:, k * _EH + g * 128: k * _EH + (g + 1) * 128],
                            rhs=xs[:, k * _TT + c0: k * _TT + c0 + w],
                            start=(k == 0),
                            stop=(k == _NK - 1),
                        )
                    z = vpool.tile([128, w], f32, tag="z")
                    nc.scalar.activation(z[:], pz[:], AF.Sigmoid,
                                         bias=bz_sb[:, g:g + 1])
                    av = vpool.tile([128, w], f32, tag="a")
                    nc.scalar.activation(av[:], z[:], AF.Copy,
                                         bias=1.0 + _EPS, scale=-1.0)
                    bv = vpool.tile([128, w], f32, tag="b")
                    nc.vector.scalar_tensor_tensor(
                        bv[:], ph[:], bh_sb[:, g:g + 1], z[:], OP.add, OP.mult
                    )
                    hb = hpool.tile([128, w], f32, tag=f"h{g}")
                    init = 0.0 if prev_ap is None else prev_ap
                    nc.vector.tensor_tensor_scan(hb[:], av[:], bv[:], init,
                                                 OP.mult, OP.add)
                    prev_ap = hb[:, w - 1:w]
                    if not last:
                        hprev[g] = hb
                    nc.sync.dma_start(
                        hout[g * 128:(g + 1) * 128,
                             t * _TT + c0: t * _TT + c0 + w], hb[:]
                    )

        if loop_n is not None:
            body(True)
            from concourse import mybir as _mb
            with tc.For_i(0, loop_n, 1, hint_engines=(_mb.EngineType.PE, _mb.EngineType.SP, _mb.EngineType.DVE, _mb.EngineType.Activation)):
                body(False)
        else:
            for rep in range(reps):
                body(rep == 0)
    nc.compile()
    return nc


_NC_CACHE = None


def _shard_inputs(inputs):
    """Per-core input maps: host pre-transposes so device DMAs are contiguous."""
    import ml_dtypes
    bf16 = ml_dtypes.bfloat16

    x = np.asarray(inputs["x"], dtype=np.float32)
    Wz = np.asarray(inputs["Wz"], dtype=np.float32)
    bz = np.asarray(inputs["bz"], dtype=np.float32)
    Wh = np.asarray(inputs["Wh"], dtype=np.float32)
    bh = np.asarray(inputs["bh"], dtype=np.float32)

    wzT = np.ascontiguousarray(Wz.T).astype(bf16)  # [d, e]
    whT = np.ascontiguousarray(Wh.T).astype(bf16)

    in_maps = []
    for i in range(8):
        b, j = i // 2, i % 2
        sl = slice(j * _EH, (j + 1) * _EH)
        in_maps.append({
            "xt": np.ascontiguousarray(x[b].T).astype(bf16),  # [D, T]
            "wzt": np.ascontiguousarray(wzT[:, sl]),          # [D, EH]
            "wht": np.ascontiguousarray(whT[:, sl]),          # [D, EH]
            "bzt": np.ascontiguousarray(bz[sl].reshape(_NG, 128).T),  # [128, NG]
            "bht": np.ascontiguousarray(bh[sl].reshape(_NG, 128).T),
        })
    return in_maps


def run(inputs, trace=False, tmpdir=None):
    """Shard inputs, run the SPMD kernel on 8 cores, reassemble full output.

    Returns (output ndarray (B, T, D) float32, BassKernelResults).
    """
    global _NC_CACHE
    from concourse.bass_utils import run_bass_kernel_spmd

    if _NC_CACHE is None:
        _NC_CACHE = _build()
    nc = _NC_CACHE

    in_maps = _shard_inputs(inputs)

    res = run_bass_kernel_spmd(
        nc, in_maps, core_ids=list(range(8)), trace=trace, tmpdir=tmpdir
    )

    out = np.empty((_B, _T, _D), dtype=np.float32)
    for i in range(8):
        b, j = i // 2, i % 2
        out[b, :, j * _EH:(j + 1) * _EH] = res.results[i]["h"].T
    return out, res


def kernel(**inputs):
    out, _ = run(inputs, trace=False)
    return out

